# revision 1
# baseline (speedup 1.0000x reference)
"""Graphwise KL loss (segment_reduce) on 8 trn2 NeuronCores.

Strategy:
  Device (the O(N) memory-bound work, data-parallel over 8 cores, each core
  streams a contiguous 1/8 slice of the element arrays):
    pr = y_true * weight
    e1 = pr * (ln(pr + 1e-37) - ln(y_pred + 1e-8))
    out: 32-element block sums of e1 and pr        (2 x 32768 f32 per core)
  Host (O(num_graphs) metadata assembly, fp64):
    Per-segment sums A_g (of e1) and B_g (of pr) are reconstructed from the
    device block sums plus fp64 partial sums of the (< 32-element) block
    prefixes at each segment boundary.  With S_g = max(B_g, EPS):
      total = mean_g (A_g - B_g * ln(S_g)) / S_g
    which equals the reference's  sum_g sum_i p*(ln p - ln q)  with
    p = pr/S_g  (identical up to the ln(max(p,EPS)) clip on the ~1e2
    elements with p < 1e-8, which contribute O(1e-7) relative error).

  Raw Bass (no Tile): this walrus build caps every non-EventSemaphore
  instruction at ONE inline sync wait, so all waits are standalone wait_ge
  instructions and all cross-engine sync is explicit, with double-buffered
  tiles (buf = t % 2) and per-engine instruction streams.
"""

import numpy as np

N_TOTAL = 8388608
N_CORES = 8
N_LOCAL = N_TOTAL // N_CORES      # 1048576
P = 128
TILE_F = 2048                     # free dim of one macro tile
TILE_ELEMS = P * TILE_F           # 262144
N_TILES = N_LOCAL // TILE_ELEMS   # 4
BLK = 32
JPT = TILE_F // BLK               # 64 block sums per partition per tile
N_BLOCKS_LOCAL = N_LOCAL // BLK   # 32768
EPS = 1e-8
TINY = 1e-37

_CACHE = {}


def _check_one_wait(nc):
    """Assert no non-EventSemaphore instruction carries more than one wait."""
    bad = []
    for f in nc.m.functions:
        for bb in f.blocks:
            for inst in bb.instructions:
                si = inst.sync_info
                if si and si.on_wait and len(si.on_wait) > 1:
                    if "EventSem" not in type(inst).__name__:
                        bad.append((type(inst).__name__, inst.name, len(si.on_wait)))
    assert not bad, f"multi-wait instructions remain: {bad}"


def _build_program():
    import concourse.bass as bass
    import concourse.mybir as mybir

    f32 = mybir.dt.float32
    Ln = mybir.ActivationFunctionType.Ln
    X = mybir.AxisListType.X
    ADD = mybir.AluOpType.add

    nc = bass.Bass()

    # Const APs for the Ln biases (same mechanism Bass.__init__ uses for 0/1).
    for val in (TINY, EPS):
        ct = nc.alloc_sbuf_tensor(f"const-f32-{val}", [128, 1], f32)
        nc.gpsimd.memset(ct.ap(), val)
        nc.const_aps.aps[(f32, val)] = ct.ap()
    nc.all_engine_barrier()

    yp = nc.declare_dram_parameter("yp", [N_LOCAL], f32, isOutput=False)
    yt = nc.declare_dram_parameter("yt", [N_LOCAL], f32, isOutput=False)
    w = nc.declare_dram_parameter("w", [N_LOCAL], f32, isOutput=False)
    o1 = nc.declare_dram_parameter("o1", [N_BLOCKS_LOCAL], f32, isOutput=True)
    o2 = nc.declare_dram_parameter("o2", [N_BLOCKS_LOCAL], f32, isOutput=True)

    yp3 = yp[:].rearrange("(t p f) -> t p f", p=P, f=TILE_F)
    yt3 = yt[:].rearrange("(t p f) -> t p f", p=P, f=TILE_F)
    w3 = w[:].rearrange("(t p f) -> t p f", p=P, f=TILE_F)
    o13 = o1[:].rearrange("(t p j) -> t p j", p=P, j=JPT)
    o23 = o2[:].rearrange("(t p j) -> t p j", p=P, j=JPT)

    # Double-buffered SBUF tiles.
    def buf2(name, shape):
        return [nc.alloc_sbuf_tensor(f"{name}{i}", shape, f32).ap() for i in range(2)]

    t_yp = buf2("t_yp", [P, TILE_F])
    t_yt = buf2("t_yt", [P, TILE_F])
    t_w = buf2("t_w", [P, TILE_F])
    t_pr = buf2("t_pr", [P, TILE_F])
    t_lp = buf2("t_lp", [P, TILE_F])
    t_lq = buf2("t_lq", [P, TILE_F])
    t_d = buf2("t_d", [P, TILE_F])
    t_e1 = buf2("t_e1", [P, TILE_F])
    t_b1 = buf2("t_b1", [P, JPT])
    t_b2 = buf2("t_b2", [P, JPT])

    # Even/odd semaphores per DMA stream: at most ONE DMA in flight per sem,
    # so its 16 completion sub-increments can't interleave with another
    # transfer's (CoreSim SemaphoreRace otherwise).
    s_yp = [nc.alloc_semaphore(f"s_yp{i}") for i in range(2)]  # +16 per load
    s_yt = [nc.alloc_semaphore(f"s_yt{i}") for i in range(2)]
    s_w = [nc.alloc_semaphore(f"s_w{i}") for i in range(2)]
    s_out = [nc.alloc_semaphore(f"s_out{i}") for i in range(2)]  # +32 per iter
    s_act = nc.alloc_semaphore("s_act")  # +1 per ACT op (lp, lq per iter)
    s_dve = nc.alloc_semaphore("s_dve")  # +1 per DVE op

    # DVE op order (hoisted pr for cross-engine overlap):
    #   pr(0), pr(1), [d,e1,r1,r2](0), pr(2), [d,e1,r1,r2](1), pr(3),
    #   [d,e1,r1,r2](2), [d,e1,r1,r2](3)
    # Absolute DVE indices (1-based):
    dve_idx = {}
    n = 0
    order = [("pr", 0), ("pr", 1)]
    for t in range(N_TILES):
        order.append(("blk", t))
        if t + 2 < N_TILES:
            order.append(("pr", t + 2))
    for item in order:
        kind, t = item
        if kind == "pr":
            n += 1
            dve_idx[("pr", t)] = n
        else:
            for opname in ("d", "e1", "r1", "r2"):
                n += 1
                dve_idx[(opname, t)] = n
    n_dve_total = n

    with nc.Block() as block:

        @block.gpsimd
        def _(g):
            for t in range(N_TILES):
                if t >= 2:
                    # typ[buf] was read by lq(t-2) = ACT op 2(t-2)+2
                    g.wait_ge(s_act, 2 * (t - 2) + 2)
                    # tyt/tw[buf] read by pr(t-2); b-out wait below covers DVE
                    g.wait_ge(s_dve, dve_idx[("pr", t - 2)])
                buf = t % 2
                g.dma_start(t_yp[buf], yp3[t, :, :]).then_inc(s_yp[buf], 16)
                g.dma_start(t_yt[buf], yt3[t, :, :]).then_inc(s_yt[buf], 16)
                g.dma_start(t_w[buf], w3[t, :, :]).then_inc(s_w[buf], 16)
                if t >= 1:
                    # store iteration t-1 outputs
                    tt = t - 1
                    g.wait_ge(s_dve, dve_idx[("r2", tt)])
                    g.dma_start(o13[tt, :, :], t_b1[tt % 2]).then_inc(s_out[tt % 2], 16)
                    g.dma_start(o23[tt, :, :], t_b2[tt % 2]).then_inc(s_out[tt % 2], 16)
            tt = N_TILES - 1
            g.wait_ge(s_dve, dve_idx[("r2", tt)])
            g.dma_start(o13[tt, :, :], t_b1[tt % 2]).then_inc(s_out[tt % 2], 16)
            g.dma_start(o23[tt, :, :], t_b2[tt % 2]).then_inc(s_out[tt % 2], 16)
            # ensure all stores landed before program end
            for i in range(2):
                g.wait_ge(s_out[i], 32 * (N_TILES // 2))

        @block.scalar
        def _(s):
            for t in range(N_TILES):
                buf = t % 2
                # lp(t) = Ln(pr(t) + TINY): needs DVE pr(t); also covers
                # lp/lq[buf] slot reuse (d(t-2) precedes pr(t) in DVE order)
                s.wait_ge(s_dve, dve_idx[("pr", t)])
                s.activation(t_lp[buf], t_pr[buf], Ln, bias=TINY).then_inc(s_act, 1)
                # lq(t) = Ln(yp(t) + EPS)
                s.wait_ge(s_yp[buf], 16 * (t // 2 + 1))
                s.activation(t_lq[buf], t_yp[buf], Ln, bias=EPS).then_inc(s_act, 1)

        @block.vector
        def _(v):
            def emit_pr(t):
                buf = t % 2
                v.wait_ge(s_yt[buf], 16 * (t // 2 + 1))
                v.wait_ge(s_w[buf], 16 * (t // 2 + 1))
                v.tensor_mul(t_pr[buf], t_yt[buf], t_w[buf]).then_inc(s_dve, 1)

            def emit_blk(t):
                buf = t % 2
                v.wait_ge(s_act, 2 * t + 2)  # lp(t), lq(t) done
                v.tensor_sub(t_d[buf], t_lp[buf], t_lq[buf]).then_inc(s_dve, 1)
                # same-engine RAW: the DVE pipeline does not forward; an op
                # reading the previous op's output needs an explicit wait
                v.wait_ge(s_dve, dve_idx[("d", t)])
                v.tensor_mul(t_e1[buf], t_pr[buf], t_d[buf]).then_inc(s_dve, 1)
                if t >= 2:
                    # b1/b2[buf] were stored by out-DMAs of t-2
                    v.wait_ge(s_out[t % 2], 32 * ((t - 2) // 2 + 1))
                v.wait_ge(s_dve, dve_idx[("e1", t)])
                v.tensor_reduce(
                    t_b1[buf], t_e1[buf].rearrange("p (j b) -> p j b", b=BLK),
                    axis=X, op=ADD,
                ).then_inc(s_dve, 1)
                v.tensor_reduce(
                    t_b2[buf], t_pr[buf].rearrange("p (j b) -> p j b", b=BLK),
                    axis=X, op=ADD,
                ).then_inc(s_dve, 1)

            for item in order:
                if item[0] == "pr":
                    emit_pr(item[1])
                else:
                    emit_blk(item[1])

    _check_one_wait(nc)
    return nc


def _get_program():
    if "nc" not in _CACHE:
        _CACHE["nc"] = _build_program()
    return _CACHE["nc"]


def _run_device(yp, yt, w, trace=False):
    from concourse.bass_utils import run_bass_kernel_spmd

    nc = _get_program()
    in_maps = [
        {
            "yp": yp[k * N_LOCAL : (k + 1) * N_LOCAL],
            "yt": yt[k * N_LOCAL : (k + 1) * N_LOCAL],
            "w": w[k * N_LOCAL : (k + 1) * N_LOCAL],
        }
        for k in range(N_CORES)
    ]
    res = run_bass_kernel_spmd(nc, in_maps, list(range(N_CORES)), trace=trace)
    bs1 = np.concatenate([r["o1"].reshape(-1) for r in res.results])
    bs2 = np.concatenate([r["o2"].reshape(-1) for r in res.results])
    return bs1, bs2, res


def kernel(y_pred, y_true, weight, segment_ptr, _trace=False):
    yp = np.ascontiguousarray(np.asarray(y_pred), dtype=np.float32).reshape(-1)
    yt = np.ascontiguousarray(np.asarray(y_true), dtype=np.float32).reshape(-1)
    w = np.ascontiguousarray(np.asarray(weight), dtype=np.float32).reshape(-1)
    ptr = np.asarray(segment_ptr).astype(np.int64).reshape(-1)
    n = yp.shape[0]
    G = ptr.shape[0] - 1
    assert n == N_TOTAL, f"kernel compiled for N={N_TOTAL}, got {n}"

    bs1, bs2, res = _run_device(yp, yt, w, trace=_trace)
    _CACHE["last_res"] = res

    # ---- host assembly in fp64 ----
    pre1 = np.empty(bs1.shape[0] + 1)
    pre1[0] = 0.0
    np.cumsum(bs1, dtype=np.float64, out=pre1[1:])
    pre2 = np.empty(bs2.shape[0] + 1)
    pre2[0] = 0.0
    np.cumsum(bs2, dtype=np.float64, out=pre2[1:])

    # clip ptr defensively to [0, n] (reference guarantees this range)
    ptrc = np.clip(ptr, 0, n)
    b_idx = ptrc // BLK
    r = ptrc - b_idx * BLK  # offset within block
    # fp64 partial sums over [ptr - r, ptr) for boundaries not block-aligned
    seg_off = np.concatenate([[0], np.cumsum(r)])
    tot = int(seg_off[-1])
    part1 = np.zeros(ptrc.shape[0])
    part2 = np.zeros(ptrc.shape[0])
    if tot > 0:
        idx = np.repeat(ptrc - r, r) + (np.arange(tot) - np.repeat(seg_off[:-1], r))
        pr_h = yt[idx].astype(np.float64) * w[idx].astype(np.float64)
        e1_h = pr_h * (np.log(pr_h + TINY) - np.log(yp[idx].astype(np.float64) + EPS))
        nz = r > 0
        red_idx = np.minimum(seg_off[:-1][nz], tot - 1).astype(np.int64)
        part1[nz] = np.add.reduceat(e1_h, red_idx)
        part2[nz] = np.add.reduceat(pr_h, red_idx)

    C1 = pre1[b_idx] + part1
    C2 = pre2[b_idx] + part2
    A = np.diff(C1)
    Bg = np.diff(C2)
    S = np.maximum(Bg, EPS)
    total = np.sum((A - Bg * np.log(S)) / S) / max(G, 1)
    return np.float32(total)



# revision 2
# speedup vs baseline: 1.4510x; 1.4510x over previous
"""Graphwise KL loss (segment_reduce) on 8 trn2 NeuronCores.

Strategy (v2 — fp16 streaming, DMA-bound):
  Device (data-parallel over 8 cores, each streams a contiguous 1/8 slice):
    Inputs are cast f32->fp16 *during the SWDGE DMA load* (gpsimd-initiated
    DMAs may cast), so every on-chip op runs on 2-byte data and the DVE
    processes 2 elem/cycle.  Per 128x1024 tile:
      Pool : 3 casting loads (yt, w, yp)
      DVE  : pr = yt*w ; d = lp-lq ; e1 = pr*d ; r1 = blksum32(e1) ;
             r2 = blksum32(pr)            (all fp16, 2x mode)
      ACT  : lq = Ln(yp+EPS) ; lp = Ln(pr+TINY)   (fp16 out)
    Block sums accumulate in one SBUF tile b_all[128, 2*8*32] and are stored
    to DRAM once at the end (single DMA).  Triple-buffered input tiles keep
    the DMA queues saturated; per-core floor = 12.6 MB / ~360 GB/s ~= 35 us.
  Host (O(num_graphs) metadata assembly, fp64):
    identical to v1 — per-segment sums A_g (of e1) and B_g (of pr) are
    reconstructed from 32-element block sums plus fp64 partial sums of the
    (<32-element) block prefixes at each segment boundary.  With
    S_g = max(B_g, EPS):  total = mean_g (A_g - B_g * ln(S_g)) / S_g.
    fp16 rounding of the element stream gives ~1.6e-6 relative error
    (simulated), far inside the 2e-2 gate.

  Raw Bass (no Tile): every non-EventSemaphore instruction carries at most
  ONE inline sync wait; all other waits are standalone wait_ge instructions.
"""

import numpy as np

N_TOTAL = 8388608
N_CORES = 8
N_LOCAL = N_TOTAL // N_CORES      # 1048576
P = 128
TILE_F = 1024                     # free dim of one macro tile
TILE_ELEMS = P * TILE_F           # 131072
N_TILES = N_LOCAL // TILE_ELEMS   # 8
BLK = 32
JPT = TILE_F // BLK               # 32 block sums per partition per tile
N_BLOCKS_LOCAL = N_LOCAL // BLK   # 32768
BUFS = 3
OUT_COLS = 2 * N_TILES * JPT      # 512 (first half e1 sums, second half pr)
EPS = 1e-8
TINY = 1e-37

_CACHE = {}


def _check_one_wait(nc):
    """Assert no non-EventSemaphore instruction carries more than one wait."""
    bad = []
    for f in nc.m.functions:
        for bb in f.blocks:
            for inst in bb.instructions:
                si = inst.sync_info
                if si and si.on_wait and len(si.on_wait) > 1:
                    if "EventSem" not in type(inst).__name__:
                        bad.append((type(inst).__name__, inst.name, len(si.on_wait)))
    assert not bad, f"multi-wait instructions remain: {bad}"


def _build_program():
    import concourse.bass as bass
    import concourse.mybir as mybir

    f32 = mybir.dt.float32
    f16 = mybir.dt.float16
    Ln = mybir.ActivationFunctionType.Ln
    X = mybir.AxisListType.X
    ADD = mybir.AluOpType.add

    nc = bass.Bass()

    # Const APs for the Ln biases (f32, read per-partition by ACT).
    # memset them on the idle DVE at stream head; ACT gates on s_init.
    consts = {}
    for val in (TINY, EPS):
        ct = nc.alloc_sbuf_tensor(f"const-f32-{val}", [128, 1], f32)
        nc.const_aps.aps[(f32, val)] = ct.ap()
        consts[val] = ct.ap()

    yp = nc.declare_dram_parameter("yp", [N_LOCAL], f32, isOutput=False)
    yt = nc.declare_dram_parameter("yt", [N_LOCAL], f32, isOutput=False)
    w = nc.declare_dram_parameter("w", [N_LOCAL], f32, isOutput=False)
    o = nc.declare_dram_parameter("o", [P * OUT_COLS], f16, isOutput=True)

    yp3 = yp[:].rearrange("(t p f) -> t p f", p=P, f=TILE_F)
    yt3 = yt[:].rearrange("(t p f) -> t p f", p=P, f=TILE_F)
    w3 = w[:].rearrange("(t p f) -> t p f", p=P, f=TILE_F)
    o2 = o[:].rearrange("(p f) -> p f", p=P)

    def bufn(name, shape, dt):
        return [nc.alloc_sbuf_tensor(f"{name}{i}", shape, dt).ap() for i in range(BUFS)]

    t_yp = bufn("t_yp", [P, TILE_F], f16)
    t_yt = bufn("t_yt", [P, TILE_F], f16)
    t_w = bufn("t_w", [P, TILE_F], f16)
    t_pr = bufn("t_pr", [P, TILE_F], f16)
    t_lp = bufn("t_lp", [P, TILE_F], f16)
    t_lq = bufn("t_lq", [P, TILE_F], f16)
    t_d = bufn("t_d", [P, TILE_F], f16)
    t_e1 = bufn("t_e1", [P, TILE_F], f16)
    b_all = nc.alloc_sbuf_tensor("b_all", [P, OUT_COLS], f16).ap()

    # Per-(tensor, buf) DMA-completion semaphores: at most one DMA in flight
    # per sem so its 16 completion sub-increments can't interleave.
    s_yp = [nc.alloc_semaphore(f"s_yp{i}") for i in range(BUFS)]
    s_yt = [nc.alloc_semaphore(f"s_yt{i}") for i in range(BUFS)]
    s_w = [nc.alloc_semaphore(f"s_w{i}") for i in range(BUFS)]
    s_out = nc.alloc_semaphore("s_out")
    s_init = nc.alloc_semaphore("s_init")
    s_act = nc.alloc_semaphore("s_act")  # +1 per ACT op: lq(t)=2t+1, lp(t)=2t+2
    s_dve = nc.alloc_semaphore("s_dve")  # +1 per DVE compute op

    # DVE emit order: hoist pr() by the buffer depth for cross-engine overlap.
    order = [("pr", t) for t in range(min(BUFS, N_TILES))]
    for t in range(N_TILES):
        order.append(("blk", t))
        if t + BUFS < N_TILES:
            order.append(("pr", t + BUFS))

    dve_idx = {}
    n = 0
    for kind, t in order:
        if kind == "pr":
            n += 1
            dve_idx[("pr", t)] = n
        else:
            for opname in ("d", "e1", "r1", "r2"):
                n += 1
                dve_idx[(opname, t)] = n

    with nc.Block() as block:

        @block.gpsimd
        def _(g):
            for t in range(N_TILES):
                b = t % BUFS
                if t >= BUFS:
                    # t_yt/t_w[b] last read by DVE pr(t-BUFS)
                    g.wait_ge(s_dve, dve_idx[("pr", t - BUFS)])
                    # t_yp[b] last read by ACT lq(t-BUFS)
                    g.wait_ge(s_act, 2 * (t - BUFS) + 1)
                g.dma_start(t_yt[b], yt3[t, :, :]).then_inc(s_yt[b], 16)
                g.dma_start(t_w[b], w3[t, :, :]).then_inc(s_w[b], 16)
                g.dma_start(t_yp[b], yp3[t, :, :]).then_inc(s_yp[b], 16)
            g.wait_ge(s_dve, dve_idx[("r2", N_TILES - 1)])
            g.dma_start(o2, b_all).then_inc(s_out, 16)
            g.wait_ge(s_out, 16)

        @block.scalar
        def _(s):
            s.wait_ge(s_init, 1)
            for t in range(N_TILES):
                b = t % BUFS
                if t >= BUFS:
                    # t_lq/t_lp[b] last read by DVE d(t-BUFS)
                    s.wait_ge(s_dve, dve_idx[("d", t - BUFS)])
                s.wait_ge(s_yp[b], 16 * (t // BUFS + 1))
                s.activation(t_lq[b], t_yp[b], Ln, bias=EPS).then_inc(s_act, 1)
                s.wait_ge(s_dve, dve_idx[("pr", t)])
                s.activation(t_lp[b], t_pr[b], Ln, bias=TINY).then_inc(s_act, 1)

        @block.vector
        def _(v):
            v.memset(consts[TINY], TINY)
            v.memset(consts[EPS], EPS).then_inc(s_init, 1)

            def emit_pr(t):
                b = t % BUFS
                if t >= BUFS:
                    # t_pr[b] last read by ACT lp(t-BUFS)
                    v.wait_ge(s_act, 2 * (t - BUFS) + 2)
                v.wait_ge(s_yt[b], 16 * (t // BUFS + 1))
                v.wait_ge(s_w[b], 16 * (t // BUFS + 1))
                v.tensor_mul(t_pr[b], t_yt[b], t_w[b]).then_inc(s_dve, 1)

            def emit_blk(t):
                b = t % BUFS
                v.wait_ge(s_act, 2 * t + 2)  # lp(t) (and lq(t)) done
                v.tensor_sub(t_d[b], t_lp[b], t_lq[b]).then_inc(s_dve, 1)
                # same-engine RAW: DVE pipeline does not forward; an op reading
                # the previous op's output needs an explicit wait
                v.wait_ge(s_dve, dve_idx[("d", t)])
                v.tensor_mul(t_e1[b], t_pr[b], t_d[b]).then_inc(s_dve, 1)
                v.wait_ge(s_dve, dve_idx[("e1", t)])
                v.tensor_reduce(
                    b_all[:, t * JPT : (t + 1) * JPT],
                    t_e1[b].rearrange("p (j b) -> p j b", b=BLK),
                    axis=X, op=ADD,
                ).then_inc(s_dve, 1)
                v.tensor_reduce(
                    b_all[:, N_TILES * JPT + t * JPT : N_TILES * JPT + (t + 1) * JPT],
                    t_pr[b].rearrange("p (j b) -> p j b", b=BLK),
                    axis=X, op=ADD,
                ).then_inc(s_dve, 1)

            with nc.allow_low_precision("fp16 block sums; 2e-2 tolerance"):
                for kind, t in order:
                    if kind == "pr":
                        emit_pr(t)
                    else:
                        emit_blk(t)

    _check_one_wait(nc)
    return nc


def _get_program():
    if "nc" not in _CACHE:
        _CACHE["nc"] = _build_program()
    return _CACHE["nc"]


def _run_device(yp, yt, w, trace=False):
    from concourse.bass_utils import run_bass_kernel_spmd

    nc = _get_program()
    in_maps = [
        {
            "yp": yp[k * N_LOCAL : (k + 1) * N_LOCAL],
            "yt": yt[k * N_LOCAL : (k + 1) * N_LOCAL],
            "w": w[k * N_LOCAL : (k + 1) * N_LOCAL],
        }
        for k in range(N_CORES)
    ]
    res = run_bass_kernel_spmd(nc, in_maps, list(range(N_CORES)), trace=trace)
    bs1 = []
    bs2 = []
    for r in res.results:
        oc = np.asarray(r["o"]).reshape(P, OUT_COLS)
        half = N_TILES * JPT
        bs1.append(oc[:, :half].reshape(P, N_TILES, JPT).transpose(1, 0, 2).reshape(-1))
        bs2.append(oc[:, half:].reshape(P, N_TILES, JPT).transpose(1, 0, 2).reshape(-1))
    return np.concatenate(bs1), np.concatenate(bs2), res


def kernel(y_pred, y_true, weight, segment_ptr, _trace=False):
    yp = np.ascontiguousarray(np.asarray(y_pred), dtype=np.float32).reshape(-1)
    yt = np.ascontiguousarray(np.asarray(y_true), dtype=np.float32).reshape(-1)
    w = np.ascontiguousarray(np.asarray(weight), dtype=np.float32).reshape(-1)
    ptr = np.asarray(segment_ptr).astype(np.int64).reshape(-1)
    n = yp.shape[0]
    G = ptr.shape[0] - 1
    assert n == N_TOTAL, f"kernel compiled for N={N_TOTAL}, got {n}"

    bs1, bs2, res = _run_device(yp, yt, w, trace=_trace)
    _CACHE["last_res"] = res

    # ---- host assembly in fp64 ----
    pre1 = np.empty(bs1.shape[0] + 1)
    pre1[0] = 0.0
    np.cumsum(bs1, dtype=np.float64, out=pre1[1:])
    pre2 = np.empty(bs2.shape[0] + 1)
    pre2[0] = 0.0
    np.cumsum(bs2, dtype=np.float64, out=pre2[1:])

    # clip ptr defensively to [0, n] (reference guarantees this range)
    ptrc = np.clip(ptr, 0, n)
    b_idx = ptrc // BLK
    r = ptrc - b_idx * BLK  # offset within block
    # fp64 partial sums over [ptr - r, ptr) for boundaries not block-aligned
    seg_off = np.concatenate([[0], np.cumsum(r)])
    tot = int(seg_off[-1])
    part1 = np.zeros(ptrc.shape[0])
    part2 = np.zeros(ptrc.shape[0])
    if tot > 0:
        idx = np.repeat(ptrc - r, r) + (np.arange(tot) - np.repeat(seg_off[:-1], r))
        pr_h = yt[idx].astype(np.float64) * w[idx].astype(np.float64)
        e1_h = pr_h * (np.log(pr_h + TINY) - np.log(yp[idx].astype(np.float64) + EPS))
        nz = r > 0
        red_idx = np.minimum(seg_off[:-1][nz], tot - 1).astype(np.int64)
        part1[nz] = np.add.reduceat(e1_h, red_idx)
        part2[nz] = np.add.reduceat(pr_h, red_idx)

    C1 = pre1[b_idx] + part1
    C2 = pre2[b_idx] + part2
    A = np.diff(C1)
    Bg = np.diff(C2)
    S = np.maximum(Bg, EPS)
    total = np.sum((A - Bg * np.log(S)) / S) / max(G, 1)
    return np.float32(total)


# revision 5
# speedup vs baseline: 1.7004x; 1.1719x over previous
"""Graphwise KL loss (segment_reduce) on 8 trn2 NeuronCores.

Strategy (v3 — fp16 streaming, single packed load per tile):
  Host packs [yt | w | yp] per tile into one contiguous f32 array, so each
  tile needs ONE gpsimd (SWDGE) dma_start that casts f32->fp16 in flight.
  Per tile (sizes graded, last tile tiny to shorten the exposed tail chain):
      DVE : pr = yt*w ; d = lp-lq ; e1 = pr*d ; r2 = blksum32(pr) ;
            r1 = blksum32(e1)                  (all fp16, TT in 2x mode)
      ACT : lq = Ln(yp+EPS) ; lp = Ln(pr+TINY) (fp16 out)
  Block sums accumulate in one SBUF tile b_all[128, 512] (fp16) and are
  stored with two small DMAs at the end.  Triple-buffered input tiles keep
  the DMA queues saturated; per-core floor = 12.6 MB / ~360 GB/s ~= 35 us.
  Host reconstructs per-segment sums from the 32-element block sums plus
  fp64 partial sums at segment boundaries (same as v1/v2); fp16 rounding
  gives ~1.6e-6 relative error vs the 2e-2 gate.

  Raw Bass (no Tile): every non-EventSemaphore instruction carries at most
  ONE inline sync wait; other waits are standalone wait_ge instructions.
"""

import numpy as np

N_TOTAL = 8388608
N_CORES = 8
N_LOCAL = N_TOTAL // N_CORES      # 1048576
P = 128
F_LIST = [1024, 1024, 1280, 1280, 1280, 1152, 960, 192]   # per-partition elems
assert sum(F_LIST) == N_LOCAL // P and all(f % 32 == 0 for f in F_LIST)
N_TILES = len(F_LIST)
F_MAX = max(F_LIST)
BLK = 32
JPT_LIST = [f // BLK for f in F_LIST]
HALF = sum(JPT_LIST)              # 256 block sums per partition per stream
OUT_COLS = 2 * HALF               # 512
N_BLOCKS_LOCAL = N_LOCAL // BLK   # 32768
BUFS = 3
EPS = 1e-8
TINY = 1e-37

_CACHE = {}


def _check_one_wait(nc):
    """Assert no non-EventSemaphore instruction carries more than one wait."""
    bad = []
    for f in nc.m.functions:
        for bb in f.blocks:
            for inst in bb.instructions:
                si = inst.sync_info
                if si and si.on_wait and len(si.on_wait) > 1:
                    if "EventSem" not in type(inst).__name__:
                        bad.append((type(inst).__name__, inst.name, len(si.on_wait)))
    assert not bad, f"multi-wait instructions remain: {bad}"


def _build_program():
    import concourse.bass as bass
    import concourse.mybir as mybir

    f32 = mybir.dt.float32
    f16 = mybir.dt.float16
    Ln = mybir.ActivationFunctionType.Ln
    X = mybir.AxisListType.X
    ADD = mybir.AluOpType.add

    nc = bass.Bass()

    # Const APs for the Ln biases (f32, read per-partition by ACT).
    # memset them on the idle DVE at stream head; ACT gates on s_init.
    consts = {}
    for val in (TINY, EPS):
        ct = nc.alloc_sbuf_tensor(f"const-f32-{val}", [128, 1], f32)
        nc.const_aps.aps[(f32, val)] = ct.ap()
        consts[val] = ct.ap()

    x = nc.declare_dram_parameter("x", [3 * N_LOCAL], f32, isOutput=False)
    o = nc.declare_dram_parameter("o", [P * OUT_COLS], f16, isOutput=True)
    o2 = o[:].rearrange("(p f) -> p f", p=P)

    # per-tile DRAM source views: [yt | w | yp] each P*F_t, tile-contiguous
    src = []
    off = 0
    for F in F_LIST:
        n = 3 * P * F
        src.append(x[off : off + n].rearrange("(c p f) -> p c f", c=3, p=P))
        off += n

    def bufn(name, cols, dt):
        return [nc.alloc_sbuf_tensor(f"{name}{i}", [P, cols], dt).ap() for i in range(BUFS)]

    t_x = bufn("t_x", 3 * F_MAX, f16)     # [yt | w | yp] fp16
    t_pr = bufn("t_pr", F_MAX, f16)
    t_lp = bufn("t_lp", F_MAX, f16)
    t_lq = bufn("t_lq", F_MAX, f16)
    t_d = bufn("t_d", F_MAX, f16)
    t_e1 = bufn("t_e1", F_MAX, f16)
    b_all = nc.alloc_sbuf_tensor("b_all", [P, OUT_COLS], f16).ap()

    # Per-buf DMA-completion semaphores: at most one DMA in flight per sem.
    s_x = [nc.alloc_semaphore(f"s_x{i}") for i in range(BUFS)]
    s_out = nc.alloc_semaphore("s_out")
    s_init = nc.alloc_semaphore("s_init")
    s_act = nc.alloc_semaphore("s_act")  # +1 per ACT op: lq(t)=2t+1, lp(t)=2t+2
    s_dve = nc.alloc_semaphore("s_dve")  # +1 per DVE compute op

    # DVE emit order: hoist pr() by the buffer depth for cross-engine overlap.
    order = [("pr", t) for t in range(min(BUFS, N_TILES))]
    for t in range(N_TILES):
        order.append(("blk", t))
        if t + BUFS < N_TILES:
            order.append(("pr", t + BUFS))

    dve_idx = {}
    n = 0
    for kind, t in order:
        if kind == "pr":
            n += 1
            dve_idx[("pr", t)] = n
        else:
            for opname in ("d", "e1", "r2", "r1"):
                n += 1
                dve_idx[(opname, t)] = n

    # b_all column offsets per tile
    c_off = [0]
    for j in JPT_LIST:
        c_off.append(c_off[-1] + j)

    with nc.Block() as block:

        @block.gpsimd
        def _(g):
            for t in range(N_TILES):
                b = t % BUFS
                if t >= BUFS:
                    # t_x[b] last read by DVE pr(t-BUFS) (yt,w) / ACT lq(t-BUFS) (yp)
                    g.wait_ge(s_dve, dve_idx[("pr", t - BUFS)])
                    g.wait_ge(s_act, 2 * (t - BUFS) + 1)
                dst = t_x[b][:, : 3 * F_LIST[t]].rearrange("p (c f) -> p c f", c=3)
                g.dma_start(dst, src[t]).then_inc(s_x[b], 16)
            g.wait_ge(s_dve, dve_idx[("r1", N_TILES - 1)])
            g.dma_start(o2[:, :HALF], b_all[:, :HALF]).then_inc(s_out, 16)
            g.dma_start(o2[:, HALF:], b_all[:, HALF:]).then_inc(s_out, 16)
            g.wait_ge(s_out, 32)

        @block.scalar
        def _(s):
            s.wait_ge(s_init, 1)
            for t in range(N_TILES):
                b = t % BUFS
                F = F_LIST[t]
                if t >= BUFS:
                    # t_lq/t_lp[b] last read by DVE d(t-BUFS)
                    s.wait_ge(s_dve, dve_idx[("d", t - BUFS)])
                s.wait_ge(s_x[b], 16 * (t // BUFS + 1))
                s.activation(
                    t_lq[b][:, :F], t_x[b][:, 2 * F : 3 * F], Ln, bias=EPS
                ).then_inc(s_act, 1)
                s.wait_ge(s_dve, dve_idx[("pr", t)])
                s.activation(t_lp[b][:, :F], t_pr[b][:, :F], Ln, bias=TINY).then_inc(
                    s_act, 1
                )

        @block.vector
        def _(v):
            v.memset(consts[TINY], TINY)
            v.memset(consts[EPS], EPS).then_inc(s_init, 1)

            def emit_pr(t):
                b = t % BUFS
                F = F_LIST[t]
                if t >= BUFS:
                    # t_pr[b] last read by ACT lp(t-BUFS)
                    v.wait_ge(s_act, 2 * (t - BUFS) + 2)
                v.wait_ge(s_x[b], 16 * (t // BUFS + 1))
                v.tensor_mul(
                    t_pr[b][:, :F], t_x[b][:, :F], t_x[b][:, F : 2 * F]
                ).then_inc(s_dve, 1)

            def emit_blk(t):
                b = t % BUFS
                F = F_LIST[t]
                J = JPT_LIST[t]
                v.wait_ge(s_act, 2 * t + 2)  # lp(t) (and lq(t)) done
                v.tensor_sub(t_d[b][:, :F], t_lp[b][:, :F], t_lq[b][:, :F]).then_inc(
                    s_dve, 1
                )
                # same-engine RAW: DVE pipeline does not forward; an op reading
                # the IMMEDIATELY previous op's output needs an explicit wait
                v.wait_ge(s_dve, dve_idx[("d", t)])
                v.tensor_mul(t_e1[b][:, :F], t_pr[b][:, :F], t_d[b][:, :F]).then_inc(
                    s_dve, 1
                )
                # r2 reads t_pr (written >=2 ops back: no wait); r1 then reads
                # t_e1 which is also >=2 ops back by emit order
                v.tensor_reduce(
                    b_all[:, HALF + c_off[t] : HALF + c_off[t + 1]],
                    t_pr[b][:, :F].rearrange("p (j b) -> p j b", b=BLK),
                    axis=X, op=ADD,
                ).then_inc(s_dve, 1)
                v.tensor_reduce(
                    b_all[:, c_off[t] : c_off[t + 1]],
                    t_e1[b][:, :F].rearrange("p (j b) -> p j b", b=BLK),
                    axis=X, op=ADD,
                ).then_inc(s_dve, 1)

            with nc.allow_low_precision("fp16 block sums; 2e-2 tolerance"):
                for kind, t in order:
                    if kind == "pr":
                        emit_pr(t)
                    else:
                        emit_blk(t)

    _check_one_wait(nc)
    return nc


def _get_program():
    if "nc" not in _CACHE:
        _CACHE["nc"] = _build_program()
    return _CACHE["nc"]


def _pack_core(yp, yt, w, k):
    """[yt | w | yp] per tile, tile-contiguous, for core k."""
    base = k * N_LOCAL
    out = np.empty(3 * N_LOCAL, dtype=np.float32)
    off = 0
    eoff = base
    for F in F_LIST:
        n = P * F
        out[off : off + n] = yt[eoff : eoff + n]
        out[off + n : off + 2 * n] = w[eoff : eoff + n]
        out[off + 2 * n : off + 3 * n] = yp[eoff : eoff + n]
        off += 3 * n
        eoff += n
    return out


def _run_device(yp, yt, w, trace=False):
    from concourse.bass_utils import run_bass_kernel_spmd

    nc = _get_program()
    in_maps = [{"x": _pack_core(yp, yt, w, k)} for k in range(N_CORES)]
    res = run_bass_kernel_spmd(nc, in_maps, list(range(N_CORES)), trace=trace)
    bs1 = []
    bs2 = []
    c_off = np.concatenate([[0], np.cumsum(JPT_LIST)]).astype(int)
    for r in res.results:
        oc = np.asarray(r["o"]).reshape(P, OUT_COLS)
        for t in range(N_TILES):
            bs1.append(oc[:, c_off[t] : c_off[t + 1]].reshape(-1))
            bs2.append(oc[:, HALF + c_off[t] : HALF + c_off[t + 1]].reshape(-1))
    return np.concatenate(bs1), np.concatenate(bs2), res


def kernel(y_pred, y_true, weight, segment_ptr, _trace=False):
    yp = np.ascontiguousarray(np.asarray(y_pred), dtype=np.float32).reshape(-1)
    yt = np.ascontiguousarray(np.asarray(y_true), dtype=np.float32).reshape(-1)
    w = np.ascontiguousarray(np.asarray(weight), dtype=np.float32).reshape(-1)
    ptr = np.asarray(segment_ptr).astype(np.int64).reshape(-1)
    n = yp.shape[0]
    G = ptr.shape[0] - 1
    assert n == N_TOTAL, f"kernel compiled for N={N_TOTAL}, got {n}"

    bs1, bs2, res = _run_device(yp, yt, w, trace=_trace)
    _CACHE["last_res"] = res

    # ---- host assembly in fp64 ----
    pre1 = np.empty(bs1.shape[0] + 1)
    pre1[0] = 0.0
    np.cumsum(bs1, dtype=np.float64, out=pre1[1:])
    pre2 = np.empty(bs2.shape[0] + 1)
    pre2[0] = 0.0
    np.cumsum(bs2, dtype=np.float64, out=pre2[1:])

    # clip ptr defensively to [0, n] (reference guarantees this range)
    ptrc = np.clip(ptr, 0, n)
    b_idx = ptrc // BLK
    r = ptrc - b_idx * BLK  # offset within block
    # fp64 partial sums over [ptr - r, ptr) for boundaries not block-aligned
    seg_off = np.concatenate([[0], np.cumsum(r)])
    tot = int(seg_off[-1])
    part1 = np.zeros(ptrc.shape[0])
    part2 = np.zeros(ptrc.shape[0])
    if tot > 0:
        idx = np.repeat(ptrc - r, r) + (np.arange(tot) - np.repeat(seg_off[:-1], r))
        pr_h = yt[idx].astype(np.float64) * w[idx].astype(np.float64)
        e1_h = pr_h * (np.log(pr_h + TINY) - np.log(yp[idx].astype(np.float64) + EPS))
        nz = r > 0
        red_idx = np.minimum(seg_off[:-1][nz], tot - 1).astype(np.int64)
        part1[nz] = np.add.reduceat(e1_h, red_idx)
        part2[nz] = np.add.reduceat(pr_h, red_idx)

    C1 = pre1[b_idx] + part1
    C2 = pre2[b_idx] + part2
    A = np.diff(C1)
    Bg = np.diff(C2)
    S = np.maximum(Bg, EPS)
    total = np.sum((A - Bg * np.log(S)) / S) / max(G, 1)
    return np.float32(total)
